# revision 28
# baseline (speedup 1.0000x reference)
"""Trainium2 Bass kernel for the CMA momentum-memory update (nn_CMA_52956946760162).

Strategy (class-sharded, full-tile packing with rotating mem stream, v3):
- Shard the C=4096 classes across 8 cores (512 classes/core), no collectives.
- Both modalities are packed into ONE chunk stream per core.  Every chunk is
  exactly 128 feature rows and <=128 psum slots; classes/segments straddling a
  chunk boundary are split, with secondary partial-sum slots merged by a tiny
  host-side add during assembly.  All device DMAs are full-128-partition
  (partial-partition DMAs starve the 16 SDMA engines in a mixed stream).
- The memory-bank rows needed for the momentum blend (valid-present segments
  and present classes, i.e. rows with blend coefficient a=0.8) form a single
  dense stream, DMAed as full [128, D] tiles decoupled from chunks.  Each
  such slot's psum partition equals its stream position mod 128, so the blend
  is two partition-ranged DVE scalar_tensor_tensor ops per chunk (split at
  the tile boundary).  a=0 slots (invalid-present -> raw mean) multiply
  whatever the mem tile holds by 0, so they may sit on any partition.
- The one-hot matrix is built on-device from 4 packed f32 coefficients per
  feature row (seg column, seg coeff, class column, class coeff) via
  iota==col compares; the feature stream carries 16B/row of metadata.
- Rows absent from the batch leave memory unchanged; the host passes them
  through from the input banks during output assembly and scatters the
  device-computed rows over them.
"""

import numpy as np

C, K, D, N = 4096, 6, 2048, 16384
SIGMA = 0.2
M = 8                 # cores
CPC = C // M          # classes per core = 512
CK = C * K
F32 = np.float32

_BUILD_CACHE = {}


class _Chunk:
    __slots__ = ("rows", "colc", "bc", "colg", "bg", "n1", "a1_src",
                 "a1_tgt", "a0_tgt", "a0_prim")

    def __init__(self):
        self.rows = []      # global feat-row ids (with modality offset)
        self.colc = []      # per row: slot ref of its seg column
        self.bc = []        # per row: seg coefficient
        self.colg = []      # per row: slot ref of its class column
        self.bg = []        # per row: class coefficient
        self.n1 = 0         # number of a!=0 slots (mem stream positions)
        self.a1_src = []    # per a!=0 slot: merged bank row id
        self.a1_tgt = []    # per a!=0 slot: merged out row id
        self.a0_tgt = []    # per a=0 slot: merged out row id
        self.a0_prim = []   # per a=0 slot: True if primary (assign), else add


def _pack_core(core, mods):
    """Pack one core's work (both modalities) into exact-128-row chunks.

    Bank row ids are encoded per modality m as base + row, where
    base = m * (CPC + CPC*K); class c -> base + c, seg s -> base + CPC + s.
    Feature row ids are encoded as m * N + row.
    Returns (chunks, targets are resolved later by the caller).
    """
    c0 = core * CPC
    chunks = [_Chunk()]
    cur = chunks[-1]
    # stream position bookkeeping happens later (per-chunk n1 + profile)

    def close():
        nonlocal cur
        chunks.append(_Chunk())
        cur = chunks[-1]

    for m, (feats, labels, cams, valid, gmem, cmem) in enumerate(mods):
        base = m * (CPC + CPC * K)
        mask = (labels >= c0) & (labels < c0 + CPC)
        rows_all = np.nonzero(mask)[0]
        lab = labels[rows_all] - c0
        seg = lab * K + cams[rows_all]
        order = np.argsort(seg, kind="stable")
        rows_all, lab, seg = rows_all[order], lab[order], seg[order]
        ccnt = np.bincount(seg, minlength=CPC * K)
        gcnt = np.bincount(lab, minlength=CPC)
        v = np.asarray(valid[c0:c0 + CPC]).reshape(CPC * K)
        class_start = np.searchsorted(lab, np.arange(CPC + 1))
        b_c_all = np.where(v, SIGMA, 1.0) / np.maximum(ccnt, 1)
        b_g_all = SIGMA / np.maximum(gcnt, 1)

        # adaptive class order: pick a row-heavy or slot-heavy class based on
        # which chunk budget is running ahead, so rows and slots fill
        # together (minimizes fragmentation -> fewer chunks)
        present = np.nonzero(gcnt > 0)[0]
        nslot_of = np.add.reduceat(
            (ccnt > 0).astype(np.int64),
            np.arange(0, CPC * K, K))[present] + 1
        bal = present[np.argsort(nslot_of - gcnt[present], kind="stable")]
        lo, hi = 0, len(bal) - 1
        while lo <= hi:
            if cur.n1 + len(cur.a0_tgt) > len(cur.rows):
                c = int(bal[lo])    # slots ahead -> take row-heavy class
                lo += 1
            else:
                c = int(bal[hi])    # rows ahead -> take slot-heavy class
                hi -= 1
            r0, r1 = int(class_start[c]), int(class_start[c + 1])
            # segments of this class: (seg_id, row_ids)
            segs = []
            i = r0
            while i < r1:
                jn = i
                while jn < r1 and seg[jn] == seg[i]:
                    jn += 1
                segs.append((int(seg[i]), rows_all[i:jn]))
                i = jn
            bg = float(b_g_all[c])
            class_primary_placed = False
            si, srow = 0, 0     # next seg index / next row within it
            while si < len(segs):
                # room check: need >= 2 slots (class col + 1 seg) and >= 1 row
                if len(cur.rows) >= 128 or \
                   cur.n1 + len(cur.a0_tgt) >= 127:
                    close()
                # place class column for this chunk
                if class_primary_placed:
                    cur.a0_tgt.append(base + c)
                    cur.a0_prim.append(False)
                    gslot_ref = ("a0", len(cur.a0_tgt) - 1)
                else:
                    cur.a1_src.append(base + c)
                    cur.a1_tgt.append(base + c)
                    gslot_ref = ("a1", cur.n1)
                    cur.n1 += 1
                    class_primary_placed = True
                placed_rows = False
                while si < len(segs):
                    s, srows = segs[si]
                    if cur.n1 + len(cur.a0_tgt) >= 128:
                        break
                    room = 128 - len(cur.rows)
                    if room == 0:
                        break
                    take = min(len(srows) - srow, room)
                    seg_primary = (srow == 0)
                    bc = float(b_c_all[s])
                    if seg_primary and v[s]:
                        cur.a1_src.append(base + CPC + s)
                        cur.a1_tgt.append(base + CPC + s)
                        sslot_ref = ("a1", cur.n1)
                        cur.n1 += 1
                    else:
                        cur.a0_tgt.append(base + CPC + s)
                        cur.a0_prim.append(seg_primary)
                        sslot_ref = ("a0", len(cur.a0_tgt) - 1)
                    for r in srows[srow:srow + take]:
                        cur.rows.append(m * N + int(r))
                        cur.colc.append(sslot_ref)
                        cur.bc.append(bc)
                        cur.colg.append(gslot_ref)
                        cur.bg.append(bg)
                    placed_rows = True
                    srow += take
                    if srow == len(srows):
                        si += 1
                        srow = 0
                    else:
                        break   # chunk rows full; seg continues next chunk
                if not placed_rows:
                    # undo the class column we just placed in this chunk
                    if gslot_ref[0] == "a0":
                        cur.a0_tgt.pop()
                        cur.a0_prim.pop()
                    else:
                        cur.a1_src.pop()
                        cur.a1_tgt.pop()
                        cur.n1 -= 1
                        class_primary_placed = False
                    close()
    if not chunks[-1].rows:
        chunks.pop()
    return chunks


def _pieces(a, b):
    """Split partition span [a, b) into spans legal for compute-engine APs
    (start 0: <=128 rows, start 32/96: <=32, start 64: <=64)."""
    out = []
    q = a
    while q < b:
        e = min(b, 64) if q == 32 else b
        out.append((q, e))
        q = e
    return out


def _build_program(prof):
    """Build + compile the SPMD Bass program.

    prof: tuple of N1_j per chunk (rows=128, slots<=128, out full tile).
    """
    import concourse.mybir as mybir
    import concourse.tile as tile
    from concourse import bacc

    f32 = mybir.dt.float32
    eq = mybir.AluOpType.is_equal
    mult = mybir.AluOpType.mult
    add = mybir.AluOpType.add
    nc = bacc.Bacc("TRN2", target_bir_lowering=False, debug=False)

    NT = len(prof)
    TM = sum(prof)
    TMT = (TM + 127) // 128          # number of full mem tiles
    MEM_BUFS = 7
    fpoh = nc.dram_tensor("fpoh", [NT * 128, D + 4], f32, kind="ExternalInput").ap()
    memin = nc.dram_tensor("memin", [TMT * 128, D], f32, kind="ExternalInput").ap()
    avec = nc.dram_tensor("avec", [128, NT], f32, kind="ExternalInput").ap()
    iota = nc.dram_tensor("iota", [128, 128], f32, kind="ExternalInput").ap()
    out = nc.dram_tensor("out", [NT * 128, D], f32, kind="ExternalOutput").ap()

    with tile.TileContext(nc) as tc:
        with tc.tile_pool(name="const", bufs=1) as constp, \
             tc.tile_pool(name="io", bufs=4) as iop, \
             tc.tile_pool(name="ps", bufs=2, space="PSUM") as psp:

            avec_t = constp.tile([128, NT], f32, name="avec_t")
            nc.sync.dma_start(out=avec_t[:], in_=avec[:, :])
            iota_t = constp.tile([128, 128], f32, name="iota_t")
            nc.sync.dma_start(out=iota_t[:], in_=iota[:, :])

            mem_tiles = {}
            loaded = 0

            def load_mem_upto(t):
                nonlocal loaded
                while loaded <= t and loaded < TMT:
                    mt = iop.tile([128, D], f32, tag="mem", bufs=MEM_BUFS,
                                  name="mem_sb")
                    nc.scalar.dma_start(
                        out=mt[:],
                        in_=memin[loaded * 128:(loaded + 1) * 128, :])
                    mem_tiles[loaded] = mt
                    loaded += 1

            S = 0
            for j, N1 in enumerate(prof):
                p = S % 128
                t = S // 128
                load_mem_upto(min(t + 4, TMT - 1))

                frow = iop.tile([128, D + 4], f32, tag="frow", bufs=6,
                                name="frow")
                nc.sync.dma_start(out=frow[:], in_=fpoh[j * 128:(j + 1) * 128, :])
                ohc = iop.tile([128, 128], f32, tag="ohc", bufs=3, name="ohc")
                oh = iop.tile([128, 128], f32, tag="oh", bufs=3, name="oh")
                nc.vector.tensor_scalar(
                    out=ohc[:], in0=iota_t[:],
                    scalar1=frow[:, D:D + 1], scalar2=frow[:, D + 1:D + 2],
                    op0=eq, op1=mult)
                nc.vector.tensor_scalar(
                    out=oh[:], in0=iota_t[:],
                    scalar1=frow[:, D + 2:D + 3], scalar2=frow[:, D + 3:D + 4],
                    op0=eq, op1=mult)
                nc.vector.scalar_tensor_tensor(
                    out=oh[:], in0=ohc[:], scalar=1.0, in1=oh[:],
                    op0=mult, op1=add)

                psum = psp.tile([128, D], f32, tag="ps", name="psum")
                for tt in range(4):
                    sl = slice(tt * 512, (tt + 1) * 512)
                    nc.tensor.matmul(psum[:, sl], oh[:], frow[:, sl],
                                     start=True, stop=True)

                mem_a = mem_tiles[t]
                mem_b = mem_tiles.get(t + 1, mem_a)
                out_sb = iop.tile([128, D], f32, tag="out", bufs=8,
                                  name="out_sb")
                # blend [p:128) from mem tile A on Vector (legal piece spans);
                # column halves let the first half drain while matmuls 2-3 run
                for q, qe in _pieces(p, 128):
                    for cs in (slice(0, D // 2), slice(D // 2, D)):
                        nc.vector.scalar_tensor_tensor(
                            out=out_sb[q:qe, cs], in0=mem_a[q:qe, cs],
                            scalar=avec_t[q:qe, j:j + 1], in1=psum[q:qe, cs],
                            op0=mult, op1=add)
                if p > 0:
                    # wrap region [0:p): only [0:w) is inside the mem window
                    # (w is 32-aligned); [w:p) is a pure PSUM drain which the
                    # Act engine handles, off Vector's critical path
                    w = max(0, p + N1 - 128)
                    for q, qe in _pieces(w, p):
                        nc.scalar.copy(out=out_sb[q:qe, :], in_=psum[q:qe, :])
                    if w > 0:
                        nc.vector.scalar_tensor_tensor(
                            out=out_sb[0:w, :], in0=mem_b[0:w, :],
                            scalar=avec_t[0:w, j:j + 1], in1=psum[0:w, :],
                            op0=mult, op1=add)
                nc.gpsimd.dma_start(out=out[j * 128:(j + 1) * 128, :],
                                    in_=out_sb[:])
                # free tiles no longer needed
                S += N1
                tnext = S // 128
                for told in [k for k in mem_tiles if k < tnext]:
                    del mem_tiles[told]

    nc.compile()
    return nc


def prepare(inputs):
    """Pack, build (or reuse) the program, and build per-core input maps."""
    a = {k: np.ascontiguousarray(np.asarray(v)) for k, v in inputs.items()}
    mods = [
        (a["rgb_feats"], a["rgb_labels"].astype(np.int64), a["rgb_cams"].astype(np.int64),
         a["vis_cam_valid"], a["vis_memory"], a["vis_cam_memory"].reshape(CK, D)),
        (a["ir_feats"], a["ir_labels"].astype(np.int64), a["ir_cams"].astype(np.int64),
         a["ir_cam_valid"], a["ir_memory"], a["ir_cam_memory"].reshape(CK, D)),
    ]
    packs = [_pack_core(core, mods) for core in range(M)]
    nch = max(len(p) for p in packs)
    # One full mem tile per chunk (stream offset p stays 0): a single
    # full-width blend op per chunk keeps Vector off the critical path.
    prof = tuple(128 for _ in range(nch))

    if prof not in _BUILD_CACHE:
        _BUILD_CACHE.clear()
        _BUILD_CACHE[prof] = _build_program(prof)
    nc = _BUILD_CACHE[prof]

    NT = len(prof)
    TM = sum(prof)
    TMT = (TM + 127) // 128
    iota_np = np.broadcast_to(np.arange(128, dtype=F32), (128, 128)).copy()
    feats_all = [mods[0][0], mods[1][0]]

    in_maps, metas = [], []
    for core in range(M):
        chunks = packs[core]
        fpoh = np.zeros((NT * 128, D + 4), F32)
        memin = np.zeros((TMT * 128, D), F32)
        avec = np.zeros((128, NT), F32)
        prim_src, prim_tgt = [], []   # out-buffer row -> global out row (assign)
        sec_src, sec_tgt = [], []     # secondary pieces (added)
        S = 0
        for j in range(NT):
            p = S % 128
            N1 = prof[j]
            if j >= len(chunks):
                S += N1
                continue
            ch = chunks[j]
            n1 = ch.n1
            a1_part = [(p + i) % 128 for i in range(n1)]
            used = set(a1_part)
            free = [q for q in range(128) if q not in used]
            assert len(ch.a0_tgt) <= len(free)
            a0_part = free[:len(ch.a0_tgt)]
            avec[a1_part, j] = 1.0 - SIGMA

            def part_of(ref):
                kind, idx = ref
                return a1_part[idx] if kind == "a1" else a0_part[idx]

            r0 = j * 128
            nr = len(ch.rows)
            rows = np.asarray(ch.rows)
            mrow = rows // N
            frow = rows % N
            for m in (0, 1):
                sel = mrow == m
                if sel.any():
                    fpoh[r0:r0 + nr, :D][sel] = feats_all[m][frow[sel]]
            fpoh[r0:r0 + nr, D] = [part_of(x) for x in ch.colc]
            fpoh[r0:r0 + nr, D + 1] = ch.bc
            fpoh[r0:r0 + nr, D + 2] = [part_of(x) for x in ch.colg]
            fpoh[r0:r0 + nr, D + 3] = ch.bg

            # mem stream rows for this chunk: positions S .. S+n1
            for i, src in enumerate(ch.a1_src):
                memin[S + i] = _bank_row(src, core, mods)
            # out targets
            for i, tgt in enumerate(ch.a1_tgt):
                prim_src.append(r0 + a1_part[i])
                prim_tgt.append(_out_row(tgt, core))
            for i, tgt in enumerate(ch.a0_tgt):
                (prim_src if ch.a0_prim[i] else sec_src).append(r0 + a0_part[i])
                (prim_tgt if ch.a0_prim[i] else sec_tgt).append(_out_row(tgt, core))
            S += N1
        in_maps.append({"fpoh": fpoh, "memin": memin, "avec": avec,
                        "iota": iota_np})
        metas.append((np.asarray(prim_src, np.int64), np.asarray(prim_tgt, np.int64),
                      np.asarray(sec_src, np.int64), np.asarray(sec_tgt, np.int64)))
    return nc, in_maps, metas, a, mods


_BANK_BASE = CPC + CPC * K


def _bank_row(src, core, mods):
    m, r = divmod(src, _BANK_BASE)
    if r < CPC:
        return mods[m][4][core * CPC + r]
    return mods[m][5][core * CPC * K + (r - CPC)]


def _out_row(tgt, core):
    m, r = divmod(tgt, _BANK_BASE)
    obase = (C + CK) * m
    if r < CPC:
        return obase + core * CPC + r
    return obase + C + core * CPC * K + (r - CPC)


def assemble(a, mods, metas, results):
    full = np.concatenate([a["vis_memory"], mods[0][5], a["ir_memory"], mods[1][5]],
                          axis=0).astype(F32, copy=True)
    for core in range(M):
        o = results[core]["out"]
        prim_src, prim_tgt, sec_src, sec_tgt = metas[core]
        full[prim_tgt] = o[prim_src]
        if len(sec_src):
            np.add.at(full, sec_tgt, o[sec_src])
    return full


def kernel(**inputs):
    from concourse.bass_utils import run_bass_kernel_spmd

    nc, in_maps, metas, a, mods = prepare(inputs)
    res = run_bass_kernel_spmd(nc, in_maps, core_ids=list(range(M)))
    return assemble(a, mods, metas, res.results)


# revision 29
# speedup vs baseline: 1.1296x; 1.1296x over previous
"""Trainium2 Bass kernel for the CMA momentum-memory update (nn_CMA_52956946760162).

Strategy (class-sharded, full-tile packing with rotating mem stream, v3):
- Shard the C=4096 classes across 8 cores (512 classes/core), no collectives.
- Both modalities are packed into ONE chunk stream per core.  Every chunk is
  exactly 128 feature rows and <=128 psum slots; classes/segments straddling a
  chunk boundary are split, with secondary partial-sum slots merged by a tiny
  host-side add during assembly.  All device DMAs are full-128-partition
  (partial-partition DMAs starve the 16 SDMA engines in a mixed stream).
- The memory-bank rows needed for the momentum blend (valid-present segments
  and present classes, i.e. rows with blend coefficient a=0.8) form a single
  dense stream, DMAed as full [128, D] tiles decoupled from chunks.  Each
  such slot's psum partition equals its stream position mod 128, so the blend
  is two partition-ranged DVE scalar_tensor_tensor ops per chunk (split at
  the tile boundary).  a=0 slots (invalid-present -> raw mean) multiply
  whatever the mem tile holds by 0, so they may sit on any partition.
- The one-hot matrix is built on-device from 4 packed f32 coefficients per
  feature row (seg column, seg coeff, class column, class coeff) via
  iota==col compares; the feature stream carries 16B/row of metadata.
- Rows absent from the batch leave memory unchanged; the host passes them
  through from the input banks during output assembly and scatters the
  device-computed rows over them.
"""

import numpy as np

C, K, D, N = 4096, 6, 2048, 16384
SIGMA = 0.2
M = 8                 # cores
CPC = C // M          # classes per core = 512
CK = C * K
F32 = np.float32

_BUILD_CACHE = {}


class _Chunk:
    __slots__ = ("rows", "colc", "bc", "colg", "bg", "n1", "a1_src",
                 "a1_tgt", "a0_tgt", "a0_prim")

    def __init__(self):
        self.rows = []      # global feat-row ids (with modality offset)
        self.colc = []      # per row: slot ref of its seg column
        self.bc = []        # per row: seg coefficient
        self.colg = []      # per row: slot ref of its class column
        self.bg = []        # per row: class coefficient
        self.n1 = 0         # number of a!=0 slots (mem stream positions)
        self.a1_src = []    # per a!=0 slot: merged bank row id
        self.a1_tgt = []    # per a!=0 slot: merged out row id
        self.a0_tgt = []    # per a=0 slot: merged out row id
        self.a0_prim = []   # per a=0 slot: True if primary (assign), else add


def _pack_core(core, mods):
    """Pack one core's work (both modalities) into exact-128-row chunks.

    Bank row ids are encoded per modality m as base + row, where
    base = m * (CPC + CPC*K); class c -> base + c, seg s -> base + CPC + s.
    Feature row ids are encoded as m * N + row.
    Returns (chunks, targets are resolved later by the caller).
    """
    c0 = core * CPC
    chunks = [_Chunk()]
    cur = chunks[-1]
    # stream position bookkeeping happens later (per-chunk n1 + profile)

    def close():
        nonlocal cur
        chunks.append(_Chunk())
        cur = chunks[-1]

    for m, (feats, labels, cams, valid, gmem, cmem) in enumerate(mods):
        base = m * (CPC + CPC * K)
        mask = (labels >= c0) & (labels < c0 + CPC)
        rows_all = np.nonzero(mask)[0]
        lab = labels[rows_all] - c0
        seg = lab * K + cams[rows_all]
        order = np.argsort(seg, kind="stable")
        rows_all, lab, seg = rows_all[order], lab[order], seg[order]
        ccnt = np.bincount(seg, minlength=CPC * K)
        gcnt = np.bincount(lab, minlength=CPC)
        v = np.asarray(valid[c0:c0 + CPC]).reshape(CPC * K)
        class_start = np.searchsorted(lab, np.arange(CPC + 1))
        b_c_all = np.where(v, SIGMA, 1.0) / np.maximum(ccnt, 1)
        b_g_all = SIGMA / np.maximum(gcnt, 1)

        # adaptive class order: pick a row-heavy or slot-heavy class based on
        # which chunk budget is running ahead, so rows and slots fill
        # together (minimizes fragmentation -> fewer chunks)
        present = np.nonzero(gcnt > 0)[0]
        nslot_of = np.add.reduceat(
            (ccnt > 0).astype(np.int64),
            np.arange(0, CPC * K, K))[present] + 1
        bal = present[np.argsort(nslot_of - gcnt[present], kind="stable")]
        lo, hi = 0, len(bal) - 1
        while lo <= hi:
            if cur.n1 + len(cur.a0_tgt) > len(cur.rows):
                c = int(bal[lo])    # slots ahead -> take row-heavy class
                lo += 1
            else:
                c = int(bal[hi])    # rows ahead -> take slot-heavy class
                hi -= 1
            r0, r1 = int(class_start[c]), int(class_start[c + 1])
            # segments of this class: (seg_id, row_ids)
            segs = []
            i = r0
            while i < r1:
                jn = i
                while jn < r1 and seg[jn] == seg[i]:
                    jn += 1
                segs.append((int(seg[i]), rows_all[i:jn]))
                i = jn
            bg = float(b_g_all[c])
            class_primary_placed = False
            si, srow = 0, 0     # next seg index / next row within it
            while si < len(segs):
                # room check: need >= 2 slots (class col + 1 seg) and >= 1 row
                if len(cur.rows) >= 128 or \
                   cur.n1 + len(cur.a0_tgt) >= 127:
                    close()
                # place class column for this chunk
                if class_primary_placed:
                    cur.a0_tgt.append(base + c)
                    cur.a0_prim.append(False)
                    gslot_ref = ("a0", len(cur.a0_tgt) - 1)
                else:
                    cur.a1_src.append(base + c)
                    cur.a1_tgt.append(base + c)
                    gslot_ref = ("a1", cur.n1)
                    cur.n1 += 1
                    class_primary_placed = True
                placed_rows = False
                while si < len(segs):
                    s, srows = segs[si]
                    if cur.n1 + len(cur.a0_tgt) >= 128:
                        break
                    room = 128 - len(cur.rows)
                    if room == 0:
                        break
                    take = min(len(srows) - srow, room)
                    seg_primary = (srow == 0)
                    bc = float(b_c_all[s])
                    if seg_primary and v[s]:
                        cur.a1_src.append(base + CPC + s)
                        cur.a1_tgt.append(base + CPC + s)
                        sslot_ref = ("a1", cur.n1)
                        cur.n1 += 1
                    else:
                        cur.a0_tgt.append(base + CPC + s)
                        cur.a0_prim.append(seg_primary)
                        sslot_ref = ("a0", len(cur.a0_tgt) - 1)
                    for r in srows[srow:srow + take]:
                        cur.rows.append(m * N + int(r))
                        cur.colc.append(sslot_ref)
                        cur.bc.append(bc)
                        cur.colg.append(gslot_ref)
                        cur.bg.append(bg)
                    placed_rows = True
                    srow += take
                    if srow == len(srows):
                        si += 1
                        srow = 0
                    else:
                        break   # chunk rows full; seg continues next chunk
                if not placed_rows:
                    # undo the class column we just placed in this chunk
                    if gslot_ref[0] == "a0":
                        cur.a0_tgt.pop()
                        cur.a0_prim.pop()
                    else:
                        cur.a1_src.pop()
                        cur.a1_tgt.pop()
                        cur.n1 -= 1
                        class_primary_placed = False
                    close()
    if not chunks[-1].rows:
        chunks.pop()
    return chunks


def _pieces(a, b):
    """Split partition span [a, b) into spans legal for compute-engine APs
    (start 0: <=128 rows, start 32/96: <=32, start 64: <=64)."""
    out = []
    q = a
    while q < b:
        e = min(b, 64) if q == 32 else b
        out.append((q, e))
        q = e
    return out


def _build_program(prof):
    """Build + compile the SPMD Bass program.

    prof: tuple of N1_j per chunk (rows=128, slots<=128, out full tile).
    """
    import concourse.mybir as mybir
    import concourse.tile as tile
    from concourse import bacc

    f32 = mybir.dt.float32
    eq = mybir.AluOpType.is_equal
    mult = mybir.AluOpType.mult
    add = mybir.AluOpType.add
    nc = bacc.Bacc("TRN2", target_bir_lowering=False, debug=False)

    NT = len(prof)
    TM = sum(prof)
    TMT = (TM + 127) // 128          # number of full mem tiles
    MEM_BUFS = 7
    fpoh = nc.dram_tensor("fpoh", [NT * 128, D + 4], f32, kind="ExternalInput").ap()
    memin = nc.dram_tensor("memin", [TMT * 128, D], f32, kind="ExternalInput").ap()
    avec = nc.dram_tensor("avec", [128, NT], f32, kind="ExternalInput").ap()
    iota = nc.dram_tensor("iota", [128, 128], f32, kind="ExternalInput").ap()
    out = nc.dram_tensor("out", [NT * 128, D], f32, kind="ExternalOutput").ap()

    with tile.TileContext(nc) as tc:
        with tc.tile_pool(name="const", bufs=1) as constp, \
             tc.tile_pool(name="io", bufs=4) as iop, \
             tc.tile_pool(name="ps", bufs=2, space="PSUM") as psp:

            avec_t = constp.tile([128, NT], f32, name="avec_t")
            nc.sync.dma_start(out=avec_t[:], in_=avec[:, :])
            iota_t = constp.tile([128, 128], f32, name="iota_t")
            nc.sync.dma_start(out=iota_t[:], in_=iota[:, :])

            mem_tiles = {}
            loaded = 0

            def load_mem_upto(t):
                nonlocal loaded
                while loaded <= t and loaded < TMT:
                    mt = iop.tile([128, D], f32, tag="mem", bufs=MEM_BUFS,
                                  name="mem_sb")
                    nc.scalar.dma_start(
                        out=mt[:],
                        in_=memin[loaded * 128:(loaded + 1) * 128, :])
                    mem_tiles[loaded] = mt
                    loaded += 1

            S = 0
            for j, N1 in enumerate(prof):
                p = S % 128
                t = S // 128
                load_mem_upto(min(t + 4, TMT - 1))

                frow = iop.tile([128, D + 4], f32, tag="frow", bufs=6,
                                name="frow")
                nc.sync.dma_start(out=frow[:], in_=fpoh[j * 128:(j + 1) * 128, :])
                ohc = iop.tile([128, 128], f32, tag="ohc", bufs=3, name="ohc")
                oh = iop.tile([128, 128], f32, tag="oh", bufs=3, name="oh")
                nc.vector.tensor_scalar(
                    out=ohc[:], in0=iota_t[:],
                    scalar1=frow[:, D:D + 1], scalar2=frow[:, D + 1:D + 2],
                    op0=eq, op1=mult)
                nc.vector.tensor_scalar(
                    out=oh[:], in0=iota_t[:],
                    scalar1=frow[:, D + 2:D + 3], scalar2=frow[:, D + 3:D + 4],
                    op0=eq, op1=mult)
                nc.vector.scalar_tensor_tensor(
                    out=oh[:], in0=ohc[:], scalar=1.0, in1=oh[:],
                    op0=mult, op1=add)

                psum = psp.tile([128, D], f32, tag="ps", name="psum")
                for tt in range(4):
                    sl = slice(tt * 512, (tt + 1) * 512)
                    nc.tensor.matmul(psum[:, sl], oh[:], frow[:, sl],
                                     start=True, stop=True)

                mem_a = mem_tiles[t]
                mem_b = mem_tiles.get(t + 1, mem_a)
                out_sb = iop.tile([128, D], f32, tag="out", bufs=8,
                                  name="out_sb")
                # blend [p:128) from mem tile A on Vector (legal piece spans)
                for q, qe in _pieces(p, 128):
                    nc.vector.scalar_tensor_tensor(
                        out=out_sb[q:qe, :], in0=mem_a[q:qe, :],
                        scalar=avec_t[q:qe, j:j + 1], in1=psum[q:qe, :],
                        op0=mult, op1=add)
                if p > 0:
                    # wrap region [0:p): only [0:w) is inside the mem window
                    # (w is 32-aligned); [w:p) is a pure PSUM drain which the
                    # Act engine handles, off Vector's critical path
                    w = max(0, p + N1 - 128)
                    for q, qe in _pieces(w, p):
                        nc.scalar.copy(out=out_sb[q:qe, :], in_=psum[q:qe, :])
                    if w > 0:
                        nc.vector.scalar_tensor_tensor(
                            out=out_sb[0:w, :], in0=mem_b[0:w, :],
                            scalar=avec_t[0:w, j:j + 1], in1=psum[0:w, :],
                            op0=mult, op1=add)
                nc.gpsimd.dma_start(out=out[j * 128:(j + 1) * 128, :],
                                    in_=out_sb[:])
                # free tiles no longer needed
                S += N1
                tnext = S // 128
                for told in [k for k in mem_tiles if k < tnext]:
                    del mem_tiles[told]

    nc.compile()
    return nc


def prepare(inputs):
    """Pack, build (or reuse) the program, and build per-core input maps."""
    a = {k: np.ascontiguousarray(np.asarray(v)) for k, v in inputs.items()}
    mods = [
        (a["rgb_feats"], a["rgb_labels"].astype(np.int64), a["rgb_cams"].astype(np.int64),
         a["vis_cam_valid"], a["vis_memory"], a["vis_cam_memory"].reshape(CK, D)),
        (a["ir_feats"], a["ir_labels"].astype(np.int64), a["ir_cams"].astype(np.int64),
         a["ir_cam_valid"], a["ir_memory"], a["ir_cam_memory"].reshape(CK, D)),
    ]
    packs = [_pack_core(core, mods) for core in range(M)]
    nch = max(len(p) for p in packs)
    # One full mem tile per chunk (stream offset p stays 0): a single
    # full-width blend op per chunk keeps Vector off the critical path.
    prof = tuple(128 for _ in range(nch))

    if prof not in _BUILD_CACHE:
        _BUILD_CACHE.clear()
        _BUILD_CACHE[prof] = _build_program(prof)
    nc = _BUILD_CACHE[prof]

    NT = len(prof)
    TM = sum(prof)
    TMT = (TM + 127) // 128
    iota_np = np.broadcast_to(np.arange(128, dtype=F32), (128, 128)).copy()
    feats_all = [mods[0][0], mods[1][0]]

    in_maps, metas = [], []
    for core in range(M):
        chunks = packs[core]
        fpoh = np.zeros((NT * 128, D + 4), F32)
        memin = np.zeros((TMT * 128, D), F32)
        avec = np.zeros((128, NT), F32)
        prim_src, prim_tgt = [], []   # out-buffer row -> global out row (assign)
        sec_src, sec_tgt = [], []     # secondary pieces (added)
        S = 0
        for j in range(NT):
            p = S % 128
            N1 = prof[j]
            if j >= len(chunks):
                S += N1
                continue
            ch = chunks[j]
            n1 = ch.n1
            a1_part = [(p + i) % 128 for i in range(n1)]
            used = set(a1_part)
            free = [q for q in range(128) if q not in used]
            assert len(ch.a0_tgt) <= len(free)
            a0_part = free[:len(ch.a0_tgt)]
            avec[a1_part, j] = 1.0 - SIGMA

            def part_of(ref):
                kind, idx = ref
                return a1_part[idx] if kind == "a1" else a0_part[idx]

            r0 = j * 128
            nr = len(ch.rows)
            rows = np.asarray(ch.rows)
            mrow = rows // N
            frow = rows % N
            for m in (0, 1):
                sel = mrow == m
                if sel.any():
                    fpoh[r0:r0 + nr, :D][sel] = feats_all[m][frow[sel]]
            fpoh[r0:r0 + nr, D] = [part_of(x) for x in ch.colc]
            fpoh[r0:r0 + nr, D + 1] = ch.bc
            fpoh[r0:r0 + nr, D + 2] = [part_of(x) for x in ch.colg]
            fpoh[r0:r0 + nr, D + 3] = ch.bg

            # mem stream rows for this chunk: positions S .. S+n1
            for i, src in enumerate(ch.a1_src):
                memin[S + i] = _bank_row(src, core, mods)
            # out targets
            for i, tgt in enumerate(ch.a1_tgt):
                prim_src.append(r0 + a1_part[i])
                prim_tgt.append(_out_row(tgt, core))
            for i, tgt in enumerate(ch.a0_tgt):
                (prim_src if ch.a0_prim[i] else sec_src).append(r0 + a0_part[i])
                (prim_tgt if ch.a0_prim[i] else sec_tgt).append(_out_row(tgt, core))
            S += N1
        in_maps.append({"fpoh": fpoh, "memin": memin, "avec": avec,
                        "iota": iota_np})
        metas.append((np.asarray(prim_src, np.int64), np.asarray(prim_tgt, np.int64),
                      np.asarray(sec_src, np.int64), np.asarray(sec_tgt, np.int64)))
    return nc, in_maps, metas, a, mods


_BANK_BASE = CPC + CPC * K


def _bank_row(src, core, mods):
    m, r = divmod(src, _BANK_BASE)
    if r < CPC:
        return mods[m][4][core * CPC + r]
    return mods[m][5][core * CPC * K + (r - CPC)]


def _out_row(tgt, core):
    m, r = divmod(tgt, _BANK_BASE)
    obase = (C + CK) * m
    if r < CPC:
        return obase + core * CPC + r
    return obase + C + core * CPC * K + (r - CPC)


def assemble(a, mods, metas, results):
    full = np.concatenate([a["vis_memory"], mods[0][5], a["ir_memory"], mods[1][5]],
                          axis=0).astype(F32, copy=True)
    for core in range(M):
        o = results[core]["out"]
        prim_src, prim_tgt, sec_src, sec_tgt = metas[core]
        full[prim_tgt] = o[prim_src]
        if len(sec_src):
            np.add.at(full, sec_tgt, o[sec_src])
    return full


def kernel(**inputs):
    from concourse.bass_utils import run_bass_kernel_spmd

    nc, in_maps, metas, a, mods = prepare(inputs)
    res = run_bass_kernel_spmd(nc, in_maps, core_ids=list(range(M)))
    return assemble(a, mods, metas, res.results)


# revision 30
# speedup vs baseline: 1.1601x; 1.0270x over previous
"""Trainium2 Bass kernel for the CMA momentum-memory update (nn_CMA_52956946760162).

Strategy (class-sharded, full-tile packing with rotating mem stream, v3):
- Shard the C=4096 classes across 8 cores (512 classes/core), no collectives.
- Both modalities are packed into ONE chunk stream per core.  Every chunk is
  exactly 128 feature rows and <=128 psum slots; classes/segments straddling a
  chunk boundary are split, with secondary partial-sum slots merged by a tiny
  host-side add during assembly.  All device DMAs are full-128-partition
  (partial-partition DMAs starve the 16 SDMA engines in a mixed stream).
- The memory-bank rows needed for the momentum blend (valid-present segments
  and present classes, i.e. rows with blend coefficient a=0.8) form a single
  dense stream, DMAed as full [128, D] tiles decoupled from chunks.  Each
  such slot's psum partition equals its stream position mod 128, so the blend
  is two partition-ranged DVE scalar_tensor_tensor ops per chunk (split at
  the tile boundary).  a=0 slots (invalid-present -> raw mean) multiply
  whatever the mem tile holds by 0, so they may sit on any partition.
- The one-hot matrix is built on-device from 4 packed f32 coefficients per
  feature row (seg column, seg coeff, class column, class coeff) via
  iota==col compares; the feature stream carries 16B/row of metadata.
- Rows absent from the batch leave memory unchanged; the host passes them
  through from the input banks during output assembly and scatters the
  device-computed rows over them.
"""

import numpy as np

C, K, D, N = 4096, 6, 2048, 16384
SIGMA = 0.2
M = 8                 # cores
CPC = C // M          # classes per core = 512
CK = C * K
F32 = np.float32

_BUILD_CACHE = {}


class _Chunk:
    __slots__ = ("rows", "colc", "bc", "colg", "bg", "n1", "a1_src",
                 "a1_tgt", "a0_tgt", "a0_prim")

    def __init__(self):
        self.rows = []      # global feat-row ids (with modality offset)
        self.colc = []      # per row: slot ref of its seg column
        self.bc = []        # per row: seg coefficient
        self.colg = []      # per row: slot ref of its class column
        self.bg = []        # per row: class coefficient
        self.n1 = 0         # number of a!=0 slots (mem stream positions)
        self.a1_src = []    # per a!=0 slot: merged bank row id
        self.a1_tgt = []    # per a!=0 slot: merged out row id
        self.a0_tgt = []    # per a=0 slot: merged out row id
        self.a0_prim = []   # per a=0 slot: True if primary (assign), else add


def _pack_core(core, mods):
    """Pack one core's work (both modalities) into exact-128-row chunks.

    Bank row ids are encoded per modality m as base + row, where
    base = m * (CPC + CPC*K); class c -> base + c, seg s -> base + CPC + s.
    Feature row ids are encoded as m * N + row.
    Returns (chunks, targets are resolved later by the caller).
    """
    c0 = core * CPC
    chunks = [_Chunk()]
    cur = chunks[-1]
    # stream position bookkeeping happens later (per-chunk n1 + profile)

    def close():
        nonlocal cur
        chunks.append(_Chunk())
        cur = chunks[-1]

    for m, (feats, labels, cams, valid, gmem, cmem) in enumerate(mods):
        base = m * (CPC + CPC * K)
        mask = (labels >= c0) & (labels < c0 + CPC)
        rows_all = np.nonzero(mask)[0]
        lab = labels[rows_all] - c0
        seg = lab * K + cams[rows_all]
        order = np.argsort(seg, kind="stable")
        rows_all, lab, seg = rows_all[order], lab[order], seg[order]
        ccnt = np.bincount(seg, minlength=CPC * K)
        gcnt = np.bincount(lab, minlength=CPC)
        v = np.asarray(valid[c0:c0 + CPC]).reshape(CPC * K)
        class_start = np.searchsorted(lab, np.arange(CPC + 1))
        b_c_all = np.where(v, SIGMA, 1.0) / np.maximum(ccnt, 1)
        b_g_all = SIGMA / np.maximum(gcnt, 1)

        # adaptive class order: pick a row-heavy or slot-heavy class based on
        # which chunk budget is running ahead, so rows and slots fill
        # together (minimizes fragmentation -> fewer chunks)
        present = np.nonzero(gcnt > 0)[0]
        nslot_of = np.add.reduceat(
            (ccnt > 0).astype(np.int64),
            np.arange(0, CPC * K, K))[present] + 1
        bal = present[np.argsort(nslot_of - gcnt[present], kind="stable")]
        lo, hi = 0, len(bal) - 1
        while lo <= hi:
            if cur.n1 + len(cur.a0_tgt) > len(cur.rows):
                c = int(bal[lo])    # slots ahead -> take row-heavy class
                lo += 1
            else:
                c = int(bal[hi])    # rows ahead -> take slot-heavy class
                hi -= 1
            r0, r1 = int(class_start[c]), int(class_start[c + 1])
            # segments of this class: (seg_id, row_ids)
            segs = []
            i = r0
            while i < r1:
                jn = i
                while jn < r1 and seg[jn] == seg[i]:
                    jn += 1
                segs.append((int(seg[i]), rows_all[i:jn]))
                i = jn
            bg = float(b_g_all[c])
            class_primary_placed = False
            si, srow = 0, 0     # next seg index / next row within it
            while si < len(segs):
                # room check: need >= 2 slots (class col + 1 seg) and >= 1 row
                if len(cur.rows) >= 128 or \
                   cur.n1 + len(cur.a0_tgt) >= 127:
                    close()
                # place class column for this chunk
                if class_primary_placed:
                    cur.a0_tgt.append(base + c)
                    cur.a0_prim.append(False)
                    gslot_ref = ("a0", len(cur.a0_tgt) - 1)
                else:
                    cur.a1_src.append(base + c)
                    cur.a1_tgt.append(base + c)
                    gslot_ref = ("a1", cur.n1)
                    cur.n1 += 1
                    class_primary_placed = True
                placed_rows = False
                while si < len(segs):
                    s, srows = segs[si]
                    if cur.n1 + len(cur.a0_tgt) >= 128:
                        break
                    room = 128 - len(cur.rows)
                    if room == 0:
                        break
                    take = min(len(srows) - srow, room)
                    seg_primary = (srow == 0)
                    bc = float(b_c_all[s])
                    if seg_primary and v[s]:
                        cur.a1_src.append(base + CPC + s)
                        cur.a1_tgt.append(base + CPC + s)
                        sslot_ref = ("a1", cur.n1)
                        cur.n1 += 1
                    else:
                        cur.a0_tgt.append(base + CPC + s)
                        cur.a0_prim.append(seg_primary)
                        sslot_ref = ("a0", len(cur.a0_tgt) - 1)
                    for r in srows[srow:srow + take]:
                        cur.rows.append(m * N + int(r))
                        cur.colc.append(sslot_ref)
                        cur.bc.append(bc)
                        cur.colg.append(gslot_ref)
                        cur.bg.append(bg)
                    placed_rows = True
                    srow += take
                    if srow == len(srows):
                        si += 1
                        srow = 0
                    else:
                        break   # chunk rows full; seg continues next chunk
                if not placed_rows:
                    # undo the class column we just placed in this chunk
                    if gslot_ref[0] == "a0":
                        cur.a0_tgt.pop()
                        cur.a0_prim.pop()
                    else:
                        cur.a1_src.pop()
                        cur.a1_tgt.pop()
                        cur.n1 -= 1
                        class_primary_placed = False
                    close()
    if not chunks[-1].rows:
        chunks.pop()
    return chunks


def _pieces(a, b):
    """Split partition span [a, b) into spans legal for compute-engine APs
    (start 0: <=128 rows, start 32/96: <=32, start 64: <=64)."""
    out = []
    q = a
    while q < b:
        e = min(b, 64) if q == 32 else b
        out.append((q, e))
        q = e
    return out


def _build_program(prof):
    """Build + compile the SPMD Bass program.

    prof: tuple of N1_j per chunk (rows=128, slots<=128, out full tile).
    """
    import concourse.mybir as mybir
    import concourse.tile as tile
    from concourse import bacc

    f32 = mybir.dt.float32
    eq = mybir.AluOpType.is_equal
    mult = mybir.AluOpType.mult
    add = mybir.AluOpType.add
    nc = bacc.Bacc("TRN2", target_bir_lowering=False, debug=False)

    NT = len(prof)
    TM = sum(prof)
    TMT = (TM + 127) // 128          # number of full mem tiles
    MEM_BUFS = 7
    fpoh = nc.dram_tensor("fpoh", [NT * 128, D + 4], f32, kind="ExternalInput").ap()
    memin = nc.dram_tensor("memin", [TMT * 128, D], f32, kind="ExternalInput").ap()
    avec = nc.dram_tensor("avec", [128, NT], f32, kind="ExternalInput").ap()
    iota = nc.dram_tensor("iota", [128, 128], f32, kind="ExternalInput").ap()
    out = nc.dram_tensor("out", [NT * 128, D], f32, kind="ExternalOutput").ap()

    with tile.TileContext(nc) as tc:
        with tc.tile_pool(name="const", bufs=1) as constp, \
             tc.tile_pool(name="io", bufs=4) as iop, \
             tc.tile_pool(name="ps", bufs=2, space="PSUM") as psp:

            avec_t = constp.tile([128, NT], f32, name="avec_t")
            nc.sync.dma_start(out=avec_t[:], in_=avec[:, :])
            iota_t = constp.tile([128, 128], f32, name="iota_t")
            nc.sync.dma_start(out=iota_t[:], in_=iota[:, :])

            mem_tiles = {}
            loaded = 0

            def load_mem_upto(t):
                nonlocal loaded
                while loaded <= t and loaded < TMT:
                    mt = iop.tile([128, D], f32, tag="mem", bufs=MEM_BUFS,
                                  name="mem_sb")
                    nc.scalar.dma_start(
                        out=mt[:],
                        in_=memin[loaded * 128:(loaded + 1) * 128, :])
                    mem_tiles[loaded] = mt
                    loaded += 1

            S = 0
            for j, N1 in enumerate(prof):
                p = S % 128
                t = S // 128
                load_mem_upto(min(t + 4, TMT - 1))

                frow = iop.tile([128, D + 4], f32, tag="frow", bufs=6,
                                name="frow")
                nc.sync.dma_start(out=frow[:], in_=fpoh[j * 128:(j + 1) * 128, :])
                ohc = iop.tile([128, 128], f32, tag="ohc", bufs=3, name="ohc")
                oh = iop.tile([128, 128], f32, tag="oh", bufs=3, name="oh")
                nc.vector.tensor_scalar(
                    out=ohc[:], in0=iota_t[:],
                    scalar1=frow[:, D:D + 1], scalar2=frow[:, D + 1:D + 2],
                    op0=eq, op1=mult)
                nc.vector.tensor_scalar(
                    out=oh[:], in0=iota_t[:],
                    scalar1=frow[:, D + 2:D + 3], scalar2=frow[:, D + 3:D + 4],
                    op0=eq, op1=mult)
                nc.vector.scalar_tensor_tensor(
                    out=oh[:], in0=ohc[:], scalar=1.0, in1=oh[:],
                    op0=mult, op1=add)

                psum = psp.tile([128, D], f32, tag="ps", name="psum")
                for tt in range(4):
                    sl = slice(tt * 512, (tt + 1) * 512)
                    nc.tensor.matmul(psum[:, sl], oh[:], frow[:, sl],
                                     start=True, stop=True)

                mem_a = mem_tiles[t]
                mem_b = mem_tiles.get(t + 1, mem_a)
                out_sb = iop.tile([128, D], f32, tag="out", bufs=8,
                                  name="out_sb")
                # blend [p:128) from mem tile A on Vector (legal piece spans)
                for q, qe in _pieces(p, 128):
                    nc.vector.scalar_tensor_tensor(
                        out=out_sb[q:qe, :], in0=mem_a[q:qe, :],
                        scalar=avec_t[q:qe, j:j + 1], in1=psum[q:qe, :],
                        op0=mult, op1=add)
                if p > 0:
                    # wrap region [0:p): only [0:w) is inside the mem window
                    # (w is 32-aligned); [w:p) is a pure PSUM drain which the
                    # Act engine handles, off Vector's critical path
                    w = max(0, p + N1 - 128)
                    for q, qe in _pieces(w, p):
                        nc.scalar.copy(out=out_sb[q:qe, :], in_=psum[q:qe, :])
                    if w > 0:
                        nc.vector.scalar_tensor_tensor(
                            out=out_sb[0:w, :], in0=mem_b[0:w, :],
                            scalar=avec_t[0:w, j:j + 1], in1=psum[0:w, :],
                            op0=mult, op1=add)
                # alternate the dependent output stream between the SWDGE and
                # Act descriptor generators to smooth issue bursts (memin
                # prefetch runs 4 chunks ahead, so Act never starves)
                out_eng = nc.gpsimd if j % 2 == 0 else nc.scalar
                out_eng.dma_start(out=out[j * 128:(j + 1) * 128, :],
                                    in_=out_sb[:])
                # free tiles no longer needed
                S += N1
                tnext = S // 128
                for told in [k for k in mem_tiles if k < tnext]:
                    del mem_tiles[told]

    nc.compile()
    return nc


def prepare(inputs):
    """Pack, build (or reuse) the program, and build per-core input maps."""
    a = {k: np.ascontiguousarray(np.asarray(v)) for k, v in inputs.items()}
    mods = [
        (a["rgb_feats"], a["rgb_labels"].astype(np.int64), a["rgb_cams"].astype(np.int64),
         a["vis_cam_valid"], a["vis_memory"], a["vis_cam_memory"].reshape(CK, D)),
        (a["ir_feats"], a["ir_labels"].astype(np.int64), a["ir_cams"].astype(np.int64),
         a["ir_cam_valid"], a["ir_memory"], a["ir_cam_memory"].reshape(CK, D)),
    ]
    packs = [_pack_core(core, mods) for core in range(M)]
    nch = max(len(p) for p in packs)
    # One full mem tile per chunk (stream offset p stays 0): a single
    # full-width blend op per chunk keeps Vector off the critical path.
    prof = tuple(128 for _ in range(nch))

    if prof not in _BUILD_CACHE:
        _BUILD_CACHE.clear()
        _BUILD_CACHE[prof] = _build_program(prof)
    nc = _BUILD_CACHE[prof]

    NT = len(prof)
    TM = sum(prof)
    TMT = (TM + 127) // 128
    iota_np = np.broadcast_to(np.arange(128, dtype=F32), (128, 128)).copy()
    feats_all = [mods[0][0], mods[1][0]]

    in_maps, metas = [], []
    for core in range(M):
        chunks = packs[core]
        fpoh = np.zeros((NT * 128, D + 4), F32)
        memin = np.zeros((TMT * 128, D), F32)
        avec = np.zeros((128, NT), F32)
        prim_src, prim_tgt = [], []   # out-buffer row -> global out row (assign)
        sec_src, sec_tgt = [], []     # secondary pieces (added)
        S = 0
        for j in range(NT):
            p = S % 128
            N1 = prof[j]
            if j >= len(chunks):
                S += N1
                continue
            ch = chunks[j]
            n1 = ch.n1
            a1_part = [(p + i) % 128 for i in range(n1)]
            used = set(a1_part)
            free = [q for q in range(128) if q not in used]
            assert len(ch.a0_tgt) <= len(free)
            a0_part = free[:len(ch.a0_tgt)]
            avec[a1_part, j] = 1.0 - SIGMA

            def part_of(ref):
                kind, idx = ref
                return a1_part[idx] if kind == "a1" else a0_part[idx]

            r0 = j * 128
            nr = len(ch.rows)
            rows = np.asarray(ch.rows)
            mrow = rows // N
            frow = rows % N
            for m in (0, 1):
                sel = mrow == m
                if sel.any():
                    fpoh[r0:r0 + nr, :D][sel] = feats_all[m][frow[sel]]
            fpoh[r0:r0 + nr, D] = [part_of(x) for x in ch.colc]
            fpoh[r0:r0 + nr, D + 1] = ch.bc
            fpoh[r0:r0 + nr, D + 2] = [part_of(x) for x in ch.colg]
            fpoh[r0:r0 + nr, D + 3] = ch.bg

            # mem stream rows for this chunk: positions S .. S+n1
            for i, src in enumerate(ch.a1_src):
                memin[S + i] = _bank_row(src, core, mods)
            # out targets
            for i, tgt in enumerate(ch.a1_tgt):
                prim_src.append(r0 + a1_part[i])
                prim_tgt.append(_out_row(tgt, core))
            for i, tgt in enumerate(ch.a0_tgt):
                (prim_src if ch.a0_prim[i] else sec_src).append(r0 + a0_part[i])
                (prim_tgt if ch.a0_prim[i] else sec_tgt).append(_out_row(tgt, core))
            S += N1
        in_maps.append({"fpoh": fpoh, "memin": memin, "avec": avec,
                        "iota": iota_np})
        metas.append((np.asarray(prim_src, np.int64), np.asarray(prim_tgt, np.int64),
                      np.asarray(sec_src, np.int64), np.asarray(sec_tgt, np.int64)))
    return nc, in_maps, metas, a, mods


_BANK_BASE = CPC + CPC * K


def _bank_row(src, core, mods):
    m, r = divmod(src, _BANK_BASE)
    if r < CPC:
        return mods[m][4][core * CPC + r]
    return mods[m][5][core * CPC * K + (r - CPC)]


def _out_row(tgt, core):
    m, r = divmod(tgt, _BANK_BASE)
    obase = (C + CK) * m
    if r < CPC:
        return obase + core * CPC + r
    return obase + C + core * CPC * K + (r - CPC)


def assemble(a, mods, metas, results):
    full = np.concatenate([a["vis_memory"], mods[0][5], a["ir_memory"], mods[1][5]],
                          axis=0).astype(F32, copy=True)
    for core in range(M):
        o = results[core]["out"]
        prim_src, prim_tgt, sec_src, sec_tgt = metas[core]
        full[prim_tgt] = o[prim_src]
        if len(sec_src):
            np.add.at(full, sec_tgt, o[sec_src])
    return full


def kernel(**inputs):
    from concourse.bass_utils import run_bass_kernel_spmd

    nc, in_maps, metas, a, mods = prepare(inputs)
    res = run_bass_kernel_spmd(nc, in_maps, core_ids=list(range(M)))
    return assemble(a, mods, metas, res.results)
